# revision 25
# baseline (speedup 1.0000x reference)
"""Trainium2 Bass kernel for nn_CatAttention (dense_transformer).

Math (per batch b, head h):
    probs* = softmax(W_*_W)           (8,8)   ConstrainedRead selectors
    Wp     = softmax(W_pred_W)        (8,64,64)
    WK[h]  = kron(probsK[h], I64)     (512,64)    (acts on d_in)
    WQ[h]  = kron(probsQ[h], I64) @ Wp[h]
    WV[h]  = kron(probsV[h], I64)
    k,q,v  = x @ W*                   (1024,64)
    S      = k @ q.T                  (p, qi)
    attn   = softmax((log(S+1e-20) + bias(qi-p)) / 8) over valid p<=qi
    out    = attn @ v

The relative-position bias table satisfies bias(d) = (1-d)/1023 for d>=1 and
bias(0) = -2.  exp(bias/8) therefore factors into a per-qi factor (cancels in
softmax), a per-p factor exp(p/8184) (folded into the Exp activation bias and
a per-p-tile constant on v), and a diagonal correction RHO applied with a tiny
Toeplitz table.  The relu(1-srow/(srow+1e-10)) correction to pre[...,0] is
<= ~1e-11 against values >= ~8 and is far below f32 resolution downstream, so
it is omitted.

S^{1/8} = exp(ln2/8 * log2(S)) is computed with the float-bit log trick:
log2(S) ~= int32_view(S)/2^23 - 127 (error in [0, 0.0861], centered).  The
Exp activation reads the score PSUM bitcast to int32 (the engine converts
int->fp32 internally) with scale = ln2/(8*2^23), eliminating the Ln pass
entirely.  Centered approximation error is +-0.37% on the attention weights,
far inside the 2e-2 budget.

x and the folded weights ship as bf16 (halves the input DMA); all matmuls
run bf16 (1 col/cycle; f32r would drop to 1/4 rate on the 65-wide attn@v
moving operand).  Scores are causally trimmed per p-tile.  The two heads of
a workgroup are processed together: their score matmuls use stationary base
partitions 0 and 64 (disjoint 64-row PE groups), so consecutive h0/h1
matmuls execute concurrently on HW, and a single 3D-AP Exp covers both
heads straight off the shared score PSUM tile.  End-to-end error vs the
fp32 reference: 3.7e-3.

Sharding: core c handles batch b=c//2 and heads 4*(c%2)..4*(c%2)+3 (two
workgroups of 2 heads stacked on the partition axis).
"""
import math
import numpy as np

BATCH, N_CTX, D_IN, N_HEADS, D_HEAD, N_VARS = 4, 1024, 512, 8, 64, 8
P = 128
NPT = N_CTX // P          # 8 p-tiles
NKT = D_IN // P           # 4 d_in tiles
QCW = 512                 # qi chunk width
NQC = N_CTX // QCW        # 2 qi chunks
INV8184 = 1.0 / (1023.0 * 8.0)
RHO = float(np.exp(np.float64(-2.0 / 8.0) - np.float64(1.0 / 1023.0 / 8.0)))
LN2 = float(np.log(np.float64(2.0)))
EXP_SCALE = LN2 / (8.0 * 2.0 ** 23)          # int32-bitcast log trick scale
RB_OFF = -127.0 * LN2 / 8.0 + 0.0861 * LN2 / 16.0   # -127*ln2/8 + centering
VA = D_HEAD + 1           # v columns + ones column
VAW = 2 * VA              # vaug width per p-tile (2 heads)

_COMPILED = {}


def _softmax_f32(w):
    w = np.asarray(w, dtype=np.float32)
    m = w.max(axis=-1, keepdims=True)
    e = np.exp(w - m, dtype=np.float32)
    return e / e.sum(axis=-1, keepdims=True, dtype=np.float32)


def _host_weights(W_K_W, W_Q_W, W_V_W, W_pred_W):
    """Fold ConstrainedRead + WPred into dense (d_in, 64) mats per head."""
    probsK = _softmax_f32(W_K_W)
    probsQ = _softmax_f32(W_Q_W)
    probsV = _softmax_f32(W_V_W)
    Wp = _softmax_f32(W_pred_W)
    eye = np.eye(D_HEAD, dtype=np.float32)
    WK = np.stack([np.kron(probsK[h][:, None], eye) for h in range(N_HEADS)])
    WQm = np.stack([np.kron(probsQ[h][:, None], eye) for h in range(N_HEADS)])
    WQ = np.einsum('hde,hef->hdf', WQm, Wp).astype(np.float32)
    WV = np.stack([np.kron(probsV[h][:, None], eye) for h in range(N_HEADS)])
    return WK, WQ, WV   # each (8, 512, 64)


def _stack_wg(W, h0, nh=2):
    """nh heads of (512,64) -> SBUF layout (128, 4, nh*64): [i, kt, m]."""
    s = np.concatenate([W[h0 + j] for j in range(nh)], axis=1)   # (512, nh*64)
    return np.ascontiguousarray(s.reshape(NKT, P, nh * D_HEAD).transpose(1, 0, 2))


def _gdiag():
    """GD[i,u] = h(u-127-i); h(d<0)=0, h(0)=RHO, h(d>0)=1."""
    i = np.arange(P)[:, None]
    u = np.arange(2 * P)[None, :]
    d = u - (P - 1) - i
    g = np.where(d < 0, 0.0, np.where(d == 0, RHO, 1.0))
    return np.ascontiguousarray(g.astype(np.float32))


def _pinned_bacc_cls():
    """Bacc that pins the ACT table set containing both Ln and Exp, so the
    Ln<->Exp alternation does not reload function tables (~1.3us each)."""
    import concourse.bacc as bacc
    import concourse.mybir as mybir
    import bass_rust as _bass_rust
    from concourse.hw_specs import get_activation_tables

    class _PinnedActBacc(bacc.Bacc):
        def insert_act_table_loads(self):
            has_activation = any(
                isinstance(i, mybir.InstActivation)
                for b in self.main_func.blocks for i in b.instructions)
            if not has_activation:
                return
            tables = [
                (name, fns if name == "natural_log_exp_and_others" else set())
                for name, fns in get_activation_tables(self.m.arch).items()
            ]
            _bass_rust.insert_act_table_loads(self, tables)

    return _PinnedActBacc


def _build_nc(reps=1, barrier=True):
    import concourse.mybir as mybir
    import concourse.tile as tile
    from contextlib import ExitStack

    F32 = mybir.dt.float32
    BF16 = mybir.dt.bfloat16

    nc = _pinned_bacc_cls()("TRN2")
    xT_d = nc.dram_tensor("xT", (P, NKT, N_CTX), BF16, kind="ExternalInput")
    WKQ_d = nc.dram_tensor("WKQ", (2, P, NKT, 2 * P), BF16, kind="ExternalInput")
    WV_d = nc.dram_tensor("WV", (P, NKT, 4 * D_HEAD), BF16, kind="ExternalInput")
    GDRB_d = nc.dram_tensor("GDRB", (P, 2 * P + 1), F32, kind="ExternalInput")
    out_d = nc.dram_tensor("out", (N_CTX, 4 * D_HEAD), F32, kind="ExternalOutput")

    EXP = mybir.ActivationFunctionType.Exp
    I32 = mybir.dt.int32

    with tile.TileContext(nc) as tc, ExitStack() as ctx:
        const_p = ctx.enter_context(tc.tile_pool(name="const", bufs=1))
        w_p = ctx.enter_context(tc.tile_pool(name="w", bufs=2))
        kq_p = ctx.enter_context(tc.tile_pool(name="kq", bufs=2))
        va_p = ctx.enter_context(tc.tile_pool(name="va", bufs=2))
        e_p = ctx.enter_context(tc.tile_pool(name="e", bufs=3))
        z_p = ctx.enter_context(tc.tile_pool(name="z", bufs=4))
        # PSUM: 8 banks of [128, 512 f32].  Scores get a 2-bank ring x2;
        # proj/vaug/mm2/warmup share a 1-bank ring x4 so attn@v never waits
        # on the score ring being freed by ACT.
        pss_p = ctx.enter_context(tc.tile_pool(name="pss", bufs=2, space="PSUM"))
        psm_p = ctx.enter_context(tc.tile_pool(name="psm", bufs=4, space="PSUM"))

        gdrb = const_p.tile([P, 2 * P + 1], F32, tag="gdrb")
        gd = gdrb[:, 0:2 * P]
        rb = gdrb[:, 2 * P:2 * P + 1]
        gd2t = const_p.tile([P, 2 * P], BF16, tag="gd2t")
        gd2 = gd2t[:, :]
        eps = const_p.tile([P, 1], F32, tag="eps")
        nc.vector.memset(eps[:], 1e-20)
        warm = const_p.tile([1, QCW], BF16, tag="warm")
        nc.vector.memset(warm[:], 0.0)

        mm2q = []
        zstages = {}

        def emit_mm2_jt(item):
            e, wg, hh, qc, vaug, jl = item
            key = (wg, qc)
            if key not in zstages:
                zstages[key] = z_p.tile([P, 4, 2 * D_HEAD], F32, tag="zst",
                                        name=f"zst_{wg}_{qc}")
            zst = zstages[key]
            jt = qc * 4 + jl
            zps = psm_p.tile([P, QCW], F32, tag="psm")
            for pt in range(jt + 1):
                nc.tensor.matmul(
                    zps[:, 0:VA],
                    e[:, hh, pt * QCW + jl * P:pt * QCW + (jl + 1) * P],
                    vaug[:, pt * 4 + wg * 2 + hh, :],
                    start=(pt == 0), stop=(pt == jt))
            rcp = z_p.tile([P, 1], F32, tag="rcp")
            nc.vector.reciprocal(rcp[:], zps[:, D_HEAD:VA])
            nc.vector.tensor_scalar_mul(
                zst[:, jl, hh * D_HEAD:(hh + 1) * D_HEAD],
                zps[:, 0:D_HEAD], rcp[:])
            if hh == 1:
                # second head staged: ship completed row blocks immediately so
                # the out DMA overlaps the remaining attn@v work.  The final
                # (wg1,qc0) tile ships as two halves to cut issue overhead on
                # the exposed tail.
                final = (wg, qc) == (1, 0)
                if not final:
                    dst = out_d[qc * QCW + jl * P:qc * QCW + (jl + 1) * P,
                                wg * 2 * D_HEAD:(wg + 1) * 2 * D_HEAD]
                    nc.sync.dma_start(dst, zst[:, jl, :])
                elif jl % 2 == 1:
                    dst = out_d[qc * QCW + (jl - 1) * P:qc * QCW + (jl + 1) * P,
                                wg * 2 * D_HEAD:(wg + 1) * 2 * D_HEAD]
                    nc.sync.dma_start(
                        dst.rearrange("(j p) c -> p j c", p=P),
                        zst[:, jl - 1:jl + 1, :])
                if jl == 3:
                    del zstages[key]

        def drain_mm2(keep):
            while len(mm2q) > keep:
                emit_mm2_jt(mm2q.pop(0))

        def drain_some(n, keep=2):
            while len(mm2q) > keep and n > 0:
                emit_mm2_jt(mm2q.pop(0))
                n -= 1

        for rep in range(reps):
          if rep and barrier:
              tc.strict_bb_all_engine_barrier()
          # PE warmup during the input-DMA wait: keeps the clock ramp (HAM)
          # from penalizing the first projections
          wps = psm_p.tile([P, QCW], F32, tag="psm")
          for _ in range(6):
              nc.tensor.matmul(wps[0:1, 0:QCW], warm[0:1, 0:1],
                               warm[0:1, :], start=True, stop=True)
          # consume the result (adds exactly 0.0 to eps) so the warmup chain
          # isn't dead-code-eliminated
          nc.vector.tensor_add(eps[0:1, 0:1], eps[0:1, 0:1], wps[0:1, 0:1])
          # wg0 weights first: the first projection needs them plus xA[kt0]
          wkq0 = w_p.tile([P, NKT, 2 * P], BF16, tag="wkq", name="wkq0")
          wkqs = {0: wkq0}
          nc.sync.dma_start(wkqs[0][:], WKQ_d[0])
          # two half-tiles so ch0 compute does not falsely depend on ch1 DMA
          xA = const_p.tile([P, NKT, QCW], BF16, tag="xA")
          xB = const_p.tile([P, NKT, QCW], BF16, tag="xB")
          nc.sync.dma_start(xA[:, 0:2, :], xT_d[:, 0:2, 0:QCW])
          nc.sync.dma_start(xA[:, 2:4, :], xT_d[:, 2:4, 0:QCW])
          if rep == 0:
              nc.sync.dma_start(gdrb[:], GDRB_d[:])
              nc.vector.tensor_copy(gd2t[:], gdrb[:, 0:2 * P])
          nc.sync.dma_start(xB[:, 0:2, :], xT_d[:, 0:2, QCW:N_CTX])
          nc.sync.dma_start(xB[:, 2:4, :], xT_d[:, 2:4, QCW:N_CTX])
          xhalf = [xA, xB]

          def xs(kt, col, width):
              t = xhalf[col // QCW]
              c = col % QCW
              return t[:, kt, c:c + width]

          wv = w_p.tile([P, NKT, 4 * D_HEAD], BF16, tag="wv")
          vaug = va_p.tile([P, NPT * 4, VA], BF16, tag="vaug")
          vaug_dma_done = [False]

          def emit_vaug(pts):
              # v projection for all 4 heads (only needed by MM2, so emitted
              # after the first k/q projections to unblock ACT sooner)
              if not vaug_dma_done[0]:
                  nc.sync.dma_start(wv[:], WV_d[:])
                  # ones columns (value c_pt) don't depend on the projection
                  for pt in range(NPT):
                      c_pt = float(math.exp(P * pt * INV8184))
                      nc.vector.memset(
                          vaug[:, pt * 4:(pt + 1) * 4, D_HEAD:VA], c_pt)
                  vaug_dma_done[0] = True
              for pt in pts:
                  vps = psm_p.tile([P, QCW], F32, tag="psm")
                  for kt in range(NKT):
                      nc.tensor.matmul(vps[:, 0:4 * D_HEAD],
                                       xs(kt, pt * P, P),
                                       wv[:, kt, :],
                                       start=(kt == 0), stop=(kt == NKT - 1))
                  c_pt = float(math.exp(P * pt * INV8184))
                  nc.vector.tensor_scalar_mul(
                      vaug[:, pt * 4:(pt + 1) * 4, 0:D_HEAD],
                      vps[:, 0:4 * D_HEAD].rearrange("p (a b) -> p a b", a=4),
                      c_pt)

          kt2s, qt2s = {}, {}

          def get_kq(wg):
              if wg not in kt2s:
                  kt2s[wg] = kq_p.tile([P, N_CTX], BF16, tag="kt2", name=f"kt2_{wg}")
                  qt2s[wg] = kq_p.tile([P, N_CTX], BF16, tag="qt2", name=f"qt2_{wg}")
              return kt2s[wg], qt2s[wg]

          def emit_proj(wg, ch, part=None):
              wkq = wkqs[wg]
              kt2, qt2 = get_kq(wg)
              cs = slice(ch * QCW, (ch + 1) * QCW)
              halves = [(kt2, wkq[:, :, 0:P]), (qt2, wkq[:, :, P:2 * P])]
              if part is not None:
                  halves = [halves[part]]
              for dst, w in halves:
                  pps = psm_p.tile([P, QCW], F32, tag="psm")
                  for kt in range(NKT):
                      nc.tensor.matmul(pps[:, 0:QCW], w[:, kt, :],
                                       xs(kt, ch * QCW, QCW),
                                       start=(kt == 0), stop=(kt == NKT - 1))
                  nc.vector.tensor_copy(dst[:, cs], pps[:, 0:QCW])

          def hook_wkq1():
              wkqs[1] = w_p.tile([P, NKT, 2 * P], BF16, tag="wkq", name="wkq1")
              nc.sync.dma_start(wkqs[1][:], WKQ_d[1])

          # PE filler work placed under the Exp windows, keyed (wg, qc, pt)
          hooks = {
              (0, 0, 0): hook_wkq1,
              (0, 0, 1): lambda: emit_vaug(range(0, 2)),
              (0, 0, 2): lambda: emit_proj(0, 1, 0),
              (0, 0, 3): lambda: emit_proj(0, 1, 1),
              (0, 1, 0): lambda: emit_vaug(range(2, 4)),
              (0, 1, 1): lambda: emit_vaug(range(4, 6)),
              (0, 1, 2): lambda: emit_vaug(range(6, 8)),
              (0, 1, 3): lambda: emit_proj(1, 0, 0),
              (0, 1, 4): lambda: emit_proj(1, 0, 1),
              (0, 1, 5): lambda: emit_proj(1, 1, 0),
              (0, 1, 6): lambda: emit_proj(1, 1, 1),
          }

          # units process both heads of a workgroup together: the two score
          # matmuls per p-tile use disjoint 64-row groups of the PE array
          # (stationary base partitions 0 and 64), so they execute
          # concurrently on HW.  (1,0) last: its exposed tail attn@v is small
          units = [(0, 0), (0, 1), (1, 1), (1, 0)]
          emit_proj(0, 0)
          for wg, qc in units:
              kt2, qt2 = get_kq(wg)
              npt = (qc + 1) * NQC * 2  # active p-tiles: 4 for qc0, 8 for qc1
              e = e_p.tile([P, 2, NPT * QCW], BF16, tag="e")
              last_unit = (wg, qc) == (1, 0)
              if last_unit:
                  # flush older attn@v first; this unit interleaves its own
                  # attn@v under its per-p-tile Exps below
                  drain_mm2(0)
              for pt in range(npt):
                  # causal trim: p-tile pt only scores against qi >= pt*P
                  off = max(0, pt * P - qc * QCW)
                  sps = pss_p.tile([P, 2, QCW], F32, tag="pss")
                  for hh in range(2):
                      hb = hh * D_HEAD
                      nc.tensor.matmul(
                          sps[:, hh, off:QCW],
                          kt2[hb:hb + D_HEAD, pt * P:(pt + 1) * P],
                          qt2[hb:hb + D_HEAD, qc * QCW + off:(qc + 1) * QCW],
                          start=True, stop=True)
                  # one Exp covers both heads (3D AP) straight off the score
                  # PSUM: the int32-bitcast log trick replaces Ln+Exp
                  nc.scalar.activation(
                      e[:, :, pt * QCW + off:(pt + 1) * QCW],
                      sps[:, :, off:QCW].bitcast(I32), EXP, bias=rb,
                      scale=EXP_SCALE)
                  dj = pt - qc * 4
                  if 0 <= dj < 4:
                      # this p-tile holds the causal diagonal at qi-block dj
                      for hh in range(2):
                          ds = slice(pt * QCW + dj * P, pt * QCW + (dj + 1) * P)
                          nc.vector.tensor_mul(e[:, hh, ds], e[:, hh, ds],
                                               gd2[:, P - 1:2 * P - 1])
                      for hh in range(2):
                          mm2q.append((e, wg, hh, qc, vaug, dj))
                  hook = hooks.pop((wg, qc, pt), None)
                  if hook is not None:
                      hook()
                  if last_unit:
                      drain_mm2(0)
                  elif (wg, qc) == (1, 1):
                      # drain the backlog early and this unit's own attn@v
                      # immediately so the tail stays small
                      drain_some(2, keep=0 if pt >= 4 else 2)
                  elif (wg, qc) == (0, 1):
                      drain_some(1)
          drain_mm2(0)
    nc.finalize()
    return nc


def _get_nc(reps=1, barrier=True):
    key = (reps, barrier)
    if key not in _COMPILED:
        _COMPILED[key] = _build_nc(reps, barrier)
    return _COMPILED[key]


def _make_runner(nc, in_maps):
    """Reusable jitted 8-core runner (no donation, device-resident inputs)."""
    import jax
    from jax.sharding import Mesh, NamedSharding, PartitionSpec
    from jax.experimental.shard_map import shard_map
    import concourse.bass2jax as b2j
    import concourse.mybir as mybir

    b2j.install_neuronx_cc_hook()
    partition_name = nc.partition_id_tensor.name if nc.partition_id_tensor else None
    in_names, out_names, out_avals, zero_outs = [], [], [], []
    for alloc in nc.m.functions[0].allocations:
        if not isinstance(alloc, mybir.MemoryLocationSet):
            continue
        name = alloc.memorylocations[0].name
        if alloc.kind == "ExternalInput":
            if name != partition_name:
                in_names.append(name)
        elif alloc.kind == "ExternalOutput":
            out_names.append(name)
            shape = tuple(alloc.tensor_shape)
            dtype = mybir.dt.np(alloc.dtype)
            out_avals.append(jax.core.ShapedArray(shape, dtype))
            zero_outs.append(np.zeros(shape, dtype))
    n_params = len(in_names)
    all_in = in_names + out_names + ([partition_name] if partition_name else [])

    def _body(*args):
        operands = list(args)
        if partition_name:
            operands.append(b2j.partition_id_tensor())
        outs = b2j._bass_exec_p.bind(
            *operands, out_avals=tuple(out_avals), in_names=tuple(all_in),
            out_names=tuple(out_names), lowering_input_output_aliases=(),
            sim_require_finite=True, sim_require_nnan=True, nc=nc)
        return tuple(outs)

    n_cores = 8
    devices = jax.devices()[:n_cores]
    mesh = Mesh(np.asarray(devices), ("core",))
    nspec = n_params + len(out_names)
    fn = jax.jit(
        shard_map(_body, mesh=mesh, in_specs=(PartitionSpec("core"),) * nspec,
                  out_specs=(PartitionSpec("core"),) * len(out_names),
                  check_rep=False),
        keep_unused=True)
    concat_in = [np.concatenate([np.asarray(in_maps[c][nm]) for c in range(n_cores)],
                                axis=0) for nm in in_names]
    concat_zeros = [np.zeros((n_cores * z.shape[0], *z.shape[1:]), z.dtype)
                    for z in zero_outs]
    sh = NamedSharding(mesh, PartitionSpec("core"))
    args = [jax.device_put(a, sh) for a in concat_in + concat_zeros]

    def run():
        outs = fn(*args)
        jax.block_until_ready(outs)
        return outs
    return run, out_names, out_avals


def _make_in_maps(x, WK, WQ, WV):
    import ml_dtypes
    bf16 = ml_dtypes.bfloat16
    gdrb = np.concatenate([
        _gdiag(),
        (np.arange(P, dtype=np.float64) * INV8184 + RB_OFF)[:, None].astype(np.float32),
    ], axis=1).astype(np.float32)
    in_maps = []
    for c in range(8):
        b, hg = c // 2, c % 2
        h0 = hg * 4
        xTh = x[b].T.reshape(NKT, P, N_CTX).transpose(1, 0, 2)
        wkq = [np.concatenate([_stack_wg(WK, h), _stack_wg(WQ, h)], axis=2)
               for h in (h0, h0 + 2)]
        in_maps.append({
            "xT": np.ascontiguousarray(xTh).astype(bf16),
            "WKQ": np.stack(wkq).astype(bf16),
            "WV": _stack_wg(WV, h0, nh=4).astype(bf16),
            "GDRB": np.ascontiguousarray(gdrb),
        })
    return in_maps


def _mask_is_tril(mask):
    mask = np.asarray(mask)
    tril = np.tril(np.ones((N_CTX, N_CTX), dtype=bool))
    return all(np.array_equal(mask[b], tril) for b in range(mask.shape[0]))


def _reference_fallback(x, mask, W_K_W, W_Q_W, W_V_W, W_pred_W):
    """Exact numpy mirror of the reference for non-causal masks."""
    x = np.asarray(x, np.float32)
    mask = np.asarray(mask, bool)
    WK, WQ, WV = _host_weights(W_K_W, W_Q_W, W_V_W, W_pred_W)
    M = N_CTX
    table = np.concatenate([
        np.array([-2.0], np.float32),
        (np.linspace(0.0, -float(M), M - 1) / M).astype(np.float32),
        (np.linspace(-float(M), 0.0, M) / M).astype(np.float32)])
    rel = (np.arange(M)[None, :] - np.arange(M)[:, None]) % (2 * M)
    bias = table[rel]
    out = np.zeros((BATCH, N_CTX, N_HEADS * D_HEAD), np.float32)
    for b in range(BATCH):
        for h in range(N_HEADS):
            k = x[b] @ WK[h]
            q = x[b] @ WQ[h]
            v = x[b] @ WV[h]
            pre = q @ k.T                                   # (qi, p)
            srow = np.where(mask[b], pre, 0.0).sum(-1)
            ms = srow / (srow + 1e-10)
            pre[:, 0] += np.maximum(1.0 - ms, 0.0)
            pos = np.log(pre + 1e-20) + bias
            masked = np.where(mask[b], pos, -1e30)
            masked = masked / 8.0
            masked -= masked.max(-1, keepdims=True)
            ex = np.exp(masked)
            attn = ex / ex.sum(-1, keepdims=True)
            out[b, :, h * 64:(h + 1) * 64] = attn @ v
    return out


def _run(inputs, trace=False, trace_kwargs=None):
    from concourse.bass_utils import run_bass_kernel_spmd
    x = np.asarray(inputs["x"], np.float32)
    WK, WQ, WV = _host_weights(inputs["W_K_W"], inputs["W_Q_W"],
                               inputs["W_V_W"], inputs["W_pred_W"])
    nc = _get_nc()
    in_maps = _make_in_maps(x, WK, WQ, WV)
    kw = {}
    if trace:
        kw = dict(trace=True, trace_kwargs=trace_kwargs or {})
    res = run_bass_kernel_spmd(nc, in_maps, list(range(8)), **kw)
    out = np.empty((BATCH, N_CTX, N_HEADS * D_HEAD), np.float32)
    for c in range(8):
        b, hg = c // 2, c % 2
        out[b, :, hg * 256:(hg + 1) * 256] = res.results[c]["out"]
    return out, res


def kernel(**inputs) -> np.ndarray:
    if not _mask_is_tril(inputs["mask"]):
        return _reference_fallback(**inputs)
    out, _ = _run(inputs)
    return out

